# revision 8
# baseline (speedup 1.0000x reference)
"""Trainium2 Bass kernel for nn_Discriminator_15668040696127.

Computes:
    q, a, d = samples[:, 0], samples[:, 1], samples[:, 2]        # [B, D]
    cos1 = <q,d> / max(||q||*||d||, 1e-6)                         # [B]
    cos2 = <a,d> / max(||a||*||d||, 1e-6)                         # [B]
    score = cos1 @ D_v1 + cos2 @ D_v2                             # scalar
    out = BCE_with_logits(score, labels[0])                       # scalar

Sharding: data-parallel over B across 8 NeuronCores (1024 samples each).
Each core computes a partial score s_c, then E_c = exp(s_c); an
on-device AllReduce with MULTIPLY yields E = prod_c E_c = exp(score) on
every core, and BCE = ln(1+E) - y*ln(E) needs only two Ln ops (same
activation table, pre-loaded by a dummy op while the collective is in
flight) on the post-collective critical path.

Tail structure (critical path after the 48 MiB stream ends):
  - last tile's d/q loads + their full epilogue hoisted to the head;
  - last tile's `a` arrives in chunks (1280,1280,1280,256): dots on DVE
    with running adds, squares of chunks 0-2 on ACT, the final [P,256]
    square on DVE so the scalar engine can pre-load the Sqrt table
    during the last chunk's DVE work;
  - cos epilogue = Sqrt activation + DVE reciprocal (single table);
  - partition reduce is one [1,1] fp32 matmul.
"""

import os
import sys

import numpy as np

for _p in ("/opt/trn_rl_repo", "/root/.axon_site/_ro/trn_rl_repo"):
    if os.path.isdir(_p) and _p not in sys.path:
        sys.path.append(_p)

import concourse.bass as bass
import concourse.bacc as bacc
import concourse.mybir as mybir
import concourse.tile as tile
from concourse import bass_utils

N_CORES = 8
B, D = 8192, 4096
BS = B // N_CORES          # 1024 samples per core
P = 128                    # SBUF partitions
T = BS // P                # 8 tiles of 128 samples per core
EPS = 1e-6

f32 = mybir.dt.float32
Alu = mybir.AluOpType
Act = mybir.ActivationFunctionType

# Last tile's `a` chunk boundaries: three big chunks, one tiny tail
# chunk so post-stream DVE work is minimal.
A_CHUNKS = [(0, 1280), (1280, 2560), (2560, 3840), (3840, 4096)]

_CACHE = {}


def _build_program():
    nc = bacc.Bacc(
        "TRN2",
        target_bir_lowering=False,
        debug=False,
        num_devices=N_CORES,
    )

    samples = nc.dram_tensor("samples", [BS, 3, D], f32, kind="ExternalInput")
    labels = nc.dram_tensor("labels", [1], f32, kind="ExternalInput")
    dv1 = nc.dram_tensor("dv1", [BS], f32, kind="ExternalInput")
    dv2 = nc.dram_tensor("dv2", [BS], f32, kind="ExternalInput")
    out = nc.dram_tensor("out", [1, 1], f32, kind="ExternalOutput")

    with tile.TileContext(nc) as tc:
        with (
            tc.tile_pool(name="data", bufs=2) as data_pool,
            tc.tile_pool(name="junk", bufs=1) as junk_pool,
            tc.tile_pool(name="stats", bufs=1) as stats_pool,
            tc.tile_pool(name="psum", bufs=1, space="PSUM") as psum_pool,
            tc.tile_pool(name="dram", bufs=1, space="DRAM") as dram_pool,
        ):
            # Interleaved stats columns: tile t owns columns 2t (q·d /
            # |q||d|) and 2t+1 (a·d / |a||d|).
            dots = stats_pool.tile([P, 2 * T], f32, tag="dots")
            nprod = stats_pool.tile([P, 2 * T], f32, tag="nprod")
            inv = stats_pool.tile([P, 2 * T], f32, tag="inv")
            contrib = stats_pool.tile([P, 2 * T], f32, tag="contrib")

            # Warm-up collective: aligns core-start skew and wakes ncfw
            # so the real collective at the tail pays less latency.
            warm = stats_pool.tile([1, 8], f32, tag="warm")
            nc.gpsimd.memset(warm[:], 0.0)
            cc_w_in = dram_pool.tile([1, 8], f32, tag="cc_w_in")
            cc_w_out = dram_pool.tile([1, 8], f32, tag="cc_w_out")
            nc.gpsimd.dma_start(cc_w_in[:], warm[:])
            nc.gpsimd.collective_compute(
                "AllReduce",
                Alu.add,
                replica_groups=[list(range(N_CORES))],
                ins=[cc_w_in[:].opt()],
                outs=[cc_w_out[:].opt()],
            )

            # Small weight/label loads up front, off the critical tail.
            # dvb column 2t holds D_v1 tile t, column 2t+1 holds D_v2.
            dvb = stats_pool.tile([P, 2 * T], f32, tag="dvb")
            ltile = stats_pool.tile([1, 1], f32, tag="ltile")
            dvb_v = dvb[:].rearrange("p (t g) -> p t g", g=2)
            nc.gpsimd.dma_start(dvb_v[:, :, 0], dv1[:].rearrange("(n p) -> p n", p=P))
            nc.gpsimd.dma_start(dvb_v[:, :, 1], dv2[:].rearrange("(n p) -> p n", p=P))
            nc.gpsimd.dma_start(ltile[:], labels[None, :])
            lneg = stats_pool.tile([1, 1], f32, tag="lneg")
            nc.vector.tensor_scalar_mul(lneg[:], ltile[:], -1.0)

            ones = stats_pool.tile([P, 1], f32, tag="ones")
            nc.gpsimd.memset(ones[:], 1.0)
            # Contiguous copy of D_v2's last-tile column for the tail's
            # fused contrib stt (scalar AP must be a standalone [P,1]).
            dvb_a = stats_pool.tile([P, 1], f32, tag="dvb_a")
            nc.gpsimd.tensor_copy(dvb_a[:], dvb[:, 2 * T - 1 : 2 * T])

            L = T - 1  # the last tile, handled out of line

            # --- Tile L's d/q loads go FIRST in the DMA queue; all the
            # q-column work for tile L happens here at the head.
            dL = stats_pool.tile([P, D], f32, tag="dL")
            qL = stats_pool.tile([P, D], f32, tag="qL")
            nc.sync.dma_start(dL[:], samples[bass.ts(L, P), 2, :])
            nc.sync.dma_start(qL[:], samples[bass.ts(L, P), 0, :])

            ddL = stats_pool.tile([P, 1], f32, tag="ddL")
            jdL = junk_pool.tile([P, D], f32, tag="junk_dve")
            nc.vector.scalar_tensor_tensor(
                out=jdL[:], in0=dL[:], scalar=1.0, in1=dL[:],
                op0=Alu.mult, op1=Alu.mult, accum_out=ddL[:],
            )
            qdL = stats_pool.tile([P, 1], f32, tag="qdL")
            jdL2 = junk_pool.tile([P, D], f32, tag="junk_dve")
            nc.vector.scalar_tensor_tensor(
                out=jdL2[:], in0=qL[:], scalar=1.0, in1=dL[:],
                op0=Alu.mult, op1=Alu.mult, accum_out=qdL[:],
            )
            qqL = stats_pool.tile([P, 1], f32, tag="qqL")
            jaL = junk_pool.tile([P, D], f32, tag="junk_act")
            nc.scalar.activation(
                out=jaL[:], in_=qL[:], func=Act.Square, accum_out=qqL[:],
            )
            # Full q-column epilogue for tile L, done at the head.
            qcol = 2 * L
            nc.vector.tensor_mul(nprod[:, qcol : qcol + 1], qqL[:], ddL[:])
            nc.scalar.activation(
                inv[:, qcol : qcol + 1], nprod[:, qcol : qcol + 1], Act.Sqrt
            )
            nc.vector.reciprocal(inv[:, qcol : qcol + 1], inv[:, qcol : qcol + 1])
            nc.vector.tensor_mul(
                contrib[:, qcol : qcol + 1], qdL[:], inv[:, qcol : qcol + 1]
            )
            nc.vector.tensor_mul(
                contrib[:, qcol : qcol + 1],
                contrib[:, qcol : qcol + 1],
                dvb[:, qcol : qcol + 1],
            )

            for t in range(T - 1):
                # Three 2 MB DMAs (d first) so compute can start as soon
                # as each component lands.
                d_t = data_pool.tile([P, D], f32, tag="d")
                q_t = data_pool.tile([P, D], f32, tag="q")
                a_t = data_pool.tile([P, D], f32, tag="a")
                nc.sync.dma_start(d_t[:], samples[bass.ts(t, P), 2, :])
                nc.sync.dma_start(q_t[:], samples[bass.ts(t, P), 0, :])
                nc.sync.dma_start(a_t[:], samples[bass.ts(t, P), 1, :])
                q, a, d = q_t[:], a_t[:], d_t[:]

                # DVE: fused product + per-partition accumulate.
                dve_accs = {}
                for src0, src1, col, atag in (
                    (d, d, None, "dd1"),
                    (q, d, 2 * t, "qd1"),
                    (a, d, 2 * t + 1, "ad1"),
                ):
                    jd = junk_pool.tile([P, D], f32, tag="junk_dve")
                    acc = junk_pool.tile([P, 1], f32, tag=atag)
                    nc.vector.scalar_tensor_tensor(
                        out=jd[:], in0=src0, scalar=1.0, in1=src1,
                        op0=Alu.mult, op1=Alu.mult, accum_out=acc[:],
                    )
                    dve_accs[atag] = acc
                    if col is not None:
                        nc.gpsimd.tensor_copy(dots[:, col : col + 1], acc[:])

                # ACT: square + accumulate for the q/a norms.
                for src0, col, atag in ((q, 2 * t, "qq1"), (a, 2 * t + 1, "aa1")):
                    ja = junk_pool.tile([P, D], f32, tag="junk_act")
                    acc = junk_pool.tile([P, 1], f32, tag=atag)
                    nc.scalar.activation(
                        out=ja[:], in_=src0, func=Act.Square, accum_out=acc[:],
                    )
                    nc.gpsimd.tensor_mul(
                        nprod[:, col : col + 1], acc[:], dve_accs["dd1"][:]
                    )

                # Per-tile epilogue: cos = dot * (1/sqrt(nprod)) * dvb.
                c2 = slice(2 * t, 2 * t + 2)
                nc.scalar.activation(inv[:, c2], nprod[:, c2], Act.Sqrt)
                nc.vector.reciprocal(inv[:, c2], inv[:, c2])
                nc.gpsimd.tensor_mul(contrib[:, c2], dots[:, c2], inv[:, c2])
                nc.gpsimd.tensor_mul(contrib[:, c2], contrib[:, c2], dvb[:, c2])

            # --- Tile L's `a` arrives last, in chunks. Dots on DVE with
            # running adds; squares of chunks 0-2 on ACT (hidden under
            # the stream); the final tiny chunk's square on DVE so the
            # scalar engine pre-loads the Sqrt table meanwhile.
            aL = stats_pool.tile([P, D], f32, tag="aL")
            ad_run = None
            aa_run = None
            nchunks = len(A_CHUNKS)
            for k, (c0, c1) in enumerate(A_CHUNKS):
                w = c1 - c0
                sl = slice(c0, c1)
                nc.sync.dma_start(aL[:, sl], samples[bass.ts(L, P), 1, sl])
                jd = junk_pool.tile([P, w], f32, tag=f"junk_adq_{w}")
                adk = junk_pool.tile([P, 1], f32, tag=f"ad_q{k}")
                nc.vector.scalar_tensor_tensor(
                    out=jd[:], in0=aL[:, sl], scalar=1.0, in1=dL[:, sl],
                    op0=Alu.mult, op1=Alu.mult, accum_out=adk[:],
                )
                aak = junk_pool.tile([P, 1], f32, tag=f"aa_q{k}")
                # ACT square (runs in parallel with DVE dots; Square
                # and Sqrt coexist in the runtime table, no reload).
                ja = junk_pool.tile([P, w], f32, tag=f"junk_aact_{w}")
                nc.scalar.activation(
                    out=ja[:], in_=aL[:, sl], func=Act.Square,
                    accum_out=aak[:],
                )
                if k == 0:
                    ad_run, aa_run = adk, aak
                else:
                    # Running adds on GpSimd, parallel with DVE dots.
                    ad_new = junk_pool.tile([P, 1], f32, tag=f"ad_r{k}")
                    aa_new = junk_pool.tile([P, 1], f32, tag=f"aa_r{k}")
                    nc.gpsimd.tensor_add(ad_new[:], ad_run[:], adk[:])
                    nc.gpsimd.tensor_add(aa_new[:], aa_run[:], aak[:])
                    ad_run, aa_run = ad_new, aa_new

            acol = 2 * L + 1
            nc.gpsimd.tensor_mul(nprod[:, acol : acol + 1], aa_run[:], ddL[:])
            nc.scalar.activation(
                inv[:, acol : acol + 1], nprod[:, acol : acol + 1], Act.Sqrt
            )
            nc.vector.reciprocal(inv[:, acol : acol + 1], inv[:, acol : acol + 1])
            # contrib_a = ad * dvb_a * inv_a in ONE fused stt.
            nc.vector.scalar_tensor_tensor(
                out=contrib[:, acol : acol + 1], in0=ad_run[:], scalar=dvb_a[:],
                op0=Alu.mult, op1=Alu.mult, in1=inv[:, acol : acol + 1],
            )

            # Row sum, then partition reduce with one [1,1] fp32 matmul.
            row_sum = stats_pool.tile([P, 1], f32, tag="row_sum")
            nc.vector.reduce_sum(row_sum[:], contrib[:], axis=mybir.AxisListType.X)
            psum_t = psum_pool.tile([1, 1], f32, tag="psum_s")
            nc.tensor.matmul(psum_t[:], row_sum[:], ones[:], start=True, stop=True)

            partial = stats_pool.tile([1, 1], f32, tag="partial")
            nc.vector.tensor_copy(partial[:], psum_t[:])

            cc_in = dram_pool.tile([1, 1], f32, tag="cc_in")
            cc_out = dram_pool.tile([1, 1], f32, tag="cc_out")
            nc.sync.dma_start(cc_in[:], partial[:])

            # Pre-load the Exp table while the AllReduce is in flight.
            exp_warm = stats_pool.tile([1, 1], f32, tag="exp_warm")
            nc.scalar.activation(exp_warm[:], ones[0:1, 0:1], Act.Exp)

            nc.gpsimd.collective_compute(
                "AllReduce",
                Alu.add,
                replica_groups=[list(range(N_CORES))],
                ins=[cc_in[:].opt()],
                outs=[cc_out[:].opt()],
            )
            red = stats_pool.tile([1, 1], f32, tag="red")
            nc.sync.dma_start(red[:], cc_out[:])
            s = red[0:1, 0:1]

            # BCE = ln(1+exp(s)) - s*y (safe for |s| << 88; score O(5)).
            exp_t = stats_pool.tile([1, 1], f32, tag="exp_t")
            sp_t = stats_pool.tile([1, 1], f32, tag="sp_t")
            bce_t = stats_pool.tile([1, 1], f32, tag="bce_t")
            nc.scalar.activation(exp_t[:], s, Act.Exp)
            nc.scalar.activation(sp_t[:], exp_t[:], Act.Ln, bias=1.0)
            nc.vector.scalar_tensor_tensor(
                out=bce_t[:], in0=s, scalar=lneg[:], in1=sp_t[:],
                op0=Alu.mult, op1=Alu.add,
            )

            nc.sync.dma_start(out[:], bce_t[:])

    nc.compile()
    return nc


def _get_program():
    if "nc" not in _CACHE:
        _CACHE["nc"] = _build_program()
    return _CACHE["nc"]


def kernel(samples, labels, D_v1, D_v2):
    samples = np.asarray(samples, dtype=np.float32)
    labels = np.asarray(labels, dtype=np.float32)
    D_v1 = np.asarray(D_v1, dtype=np.float32)
    D_v2 = np.asarray(D_v2, dtype=np.float32)
    assert samples.shape == (B, 3, D), samples.shape

    nc = _get_program()

    in_maps = []
    for c in range(N_CORES):
        sl = slice(c * BS, (c + 1) * BS)
        in_maps.append(
            {
                "samples": np.ascontiguousarray(samples[sl]),
                "labels": labels,
                "dv1": np.ascontiguousarray(D_v1[sl]),
                "dv2": np.ascontiguousarray(D_v2[sl]),
            }
        )

    _tc = os.environ.get("KERNEL_TRACE_CORES")
    _kw = {"trace_cores": [int(x) for x in _tc.split(",")]} if _tc else {}
    try:
        res = bass_utils.run_bass_kernel_spmd(
            nc, in_maps, core_ids=list(range(N_CORES)), **_kw
        )
    except Exception:
        # A previously-wedged NeuronCore surfaces as an unrecoverable
        # exec error on the first attempt; the runtime resets it, so a
        # single retry recovers.
        res = bass_utils.run_bass_kernel_spmd(
            nc, in_maps, core_ids=list(range(N_CORES)), **_kw
        )
    _CACHE["last_results"] = res
    return np.asarray(res.results[0]["out"], dtype=np.float32).reshape(())


# revision 9
# speedup vs baseline: 1.2211x; 1.2211x over previous
"""Trainium2 Bass kernel for nn_Discriminator_15668040696127.

Computes:
    q, a, d = samples[:, 0], samples[:, 1], samples[:, 2]        # [B, D]
    cos1 = <q,d> / max(||q||*||d||, 1e-6)                         # [B]
    cos2 = <a,d> / max(||a||*||d||, 1e-6)                         # [B]
    score = cos1 @ D_v1 + cos2 @ D_v2                             # scalar
    out = BCE_with_logits(score, labels[0])                       # scalar

Sharding: data-parallel over B across 8 NeuronCores (1024 samples each).
Each core computes a partial score s_c, then E_c = exp(s_c); an
on-device AllReduce with MULTIPLY yields E = prod_c E_c = exp(score) on
every core, and BCE = ln(1+E) - y*ln(E) needs only two Ln ops (same
activation table, pre-loaded by a dummy op while the collective is in
flight) on the post-collective critical path.

Tail structure (critical path after the 48 MiB stream ends):
  - last tile's d/q loads + their full epilogue hoisted to the head;
  - last tile's `a` arrives in chunks (1280,1280,1280,256): dots on DVE
    with running adds, squares of chunks 0-2 on ACT, the final [P,256]
    square on DVE so the scalar engine can pre-load the Sqrt table
    during the last chunk's DVE work;
  - cos epilogue = Sqrt activation + DVE reciprocal (single table);
  - partition reduce is one [1,1] fp32 matmul.
"""

import os
import sys

import numpy as np

for _p in ("/opt/trn_rl_repo", "/root/.axon_site/_ro/trn_rl_repo"):
    if os.path.isdir(_p) and _p not in sys.path:
        sys.path.append(_p)

import concourse.bass as bass
import concourse.bacc as bacc
import concourse.mybir as mybir
import concourse.tile as tile
from concourse import bass_utils

N_CORES = 8
B, D = 8192, 4096
BS = B // N_CORES          # 1024 samples per core
P = 128                    # SBUF partitions
T = BS // P                # 8 tiles of 128 samples per core
EPS = 1e-6

f32 = mybir.dt.float32
Alu = mybir.AluOpType
Act = mybir.ActivationFunctionType

# Last tile's `a` chunk boundaries: three big chunks, one tiny tail
# chunk so post-stream DVE work is minimal.
A_CHUNKS = [(0, 1280), (1280, 2560), (2560, 3840), (3840, 4096)]

_CACHE = {}


def _build_program():
    nc = bacc.Bacc(
        "TRN2",
        target_bir_lowering=False,
        debug=False,
        num_devices=N_CORES,
    )

    samples = nc.dram_tensor("samples", [BS, 3, D], f32, kind="ExternalInput")
    labels = nc.dram_tensor("labels", [1], f32, kind="ExternalInput")
    dv1 = nc.dram_tensor("dv1", [BS], f32, kind="ExternalInput")
    dv2 = nc.dram_tensor("dv2", [BS], f32, kind="ExternalInput")
    out = nc.dram_tensor("out", [1, 1], f32, kind="ExternalOutput")

    with tile.TileContext(nc) as tc:
        with (
            tc.tile_pool(name="data", bufs=2) as data_pool,
            tc.tile_pool(name="junk", bufs=1) as junk_pool,
            tc.tile_pool(name="stats", bufs=1) as stats_pool,
            tc.tile_pool(name="psum", bufs=1, space="PSUM") as psum_pool,
            tc.tile_pool(name="dram", bufs=1, space="DRAM") as dram_pool,
        ):
            # Interleaved stats columns: tile t owns columns 2t (q·d /
            # |q||d|) and 2t+1 (a·d / |a||d|).
            dots = stats_pool.tile([P, 2 * T], f32, tag="dots")
            nprod = stats_pool.tile([P, 2 * T], f32, tag="nprod")
            inv = stats_pool.tile([P, 2 * T], f32, tag="inv")
            contrib = stats_pool.tile([P, 2 * T], f32, tag="contrib")

            # Warm-up collective: aligns core-start skew and wakes ncfw
            # so the real collective at the tail pays less latency.
            warm = stats_pool.tile([1, 8], f32, tag="warm")
            nc.gpsimd.memset(warm[:], 0.0)
            cc_w_in = dram_pool.tile([1, 8], f32, tag="cc_w_in")
            cc_w_out = dram_pool.tile([1, 8], f32, tag="cc_w_out")
            nc.gpsimd.dma_start(cc_w_in[:], warm[:])
            nc.gpsimd.collective_compute(
                "AllReduce",
                Alu.add,
                replica_groups=[list(range(N_CORES))],
                ins=[cc_w_in[:].opt()],
                outs=[cc_w_out[:].opt()],
            )

            # Small weight/label loads up front, off the critical tail.
            # dvb column 2t holds D_v1 tile t, column 2t+1 holds D_v2.
            dvb = stats_pool.tile([P, 2 * T], f32, tag="dvb")
            ltile = stats_pool.tile([1, 1], f32, tag="ltile")
            dvb_v = dvb[:].rearrange("p (t g) -> p t g", g=2)
            nc.gpsimd.dma_start(dvb_v[:, :, 0], dv1[:].rearrange("(n p) -> p n", p=P))
            nc.gpsimd.dma_start(dvb_v[:, :, 1], dv2[:].rearrange("(n p) -> p n", p=P))
            nc.gpsimd.dma_start(ltile[:], labels[None, :])
            lneg = stats_pool.tile([1, 1], f32, tag="lneg")
            nc.vector.tensor_scalar_mul(lneg[:], ltile[:], -1.0)

            ones = stats_pool.tile([P, 1], f32, tag="ones")
            nc.gpsimd.memset(ones[:], 1.0)
            # Contiguous copy of D_v2's last-tile column for the tail's
            # fused contrib stt (scalar AP must be a standalone [P,1]).
            dvb_a = stats_pool.tile([P, 1], f32, tag="dvb_a")
            nc.gpsimd.tensor_copy(dvb_a[:], dvb[:, 2 * T - 1 : 2 * T])

            L = T - 1  # the last tile, handled out of line

            # --- Tile L's d/q loads go FIRST in the DMA queue; all the
            # q-column work for tile L happens here at the head.
            dL = stats_pool.tile([P, D], f32, tag="dL")
            qL = stats_pool.tile([P, D], f32, tag="qL")
            nc.sync.dma_start(dL[:], samples[bass.ts(L, P), 2, :])
            nc.sync.dma_start(qL[:], samples[bass.ts(L, P), 0, :])

            ddL = stats_pool.tile([P, 1], f32, tag="ddL")
            jdL = junk_pool.tile([P, D], f32, tag="junk_dve")
            nc.vector.scalar_tensor_tensor(
                out=jdL[:], in0=dL[:], scalar=1.0, in1=dL[:],
                op0=Alu.mult, op1=Alu.mult, accum_out=ddL[:],
            )
            qdL = stats_pool.tile([P, 1], f32, tag="qdL")
            jdL2 = junk_pool.tile([P, D], f32, tag="junk_dve")
            nc.vector.scalar_tensor_tensor(
                out=jdL2[:], in0=qL[:], scalar=1.0, in1=dL[:],
                op0=Alu.mult, op1=Alu.mult, accum_out=qdL[:],
            )
            qqL = stats_pool.tile([P, 1], f32, tag="qqL")
            jaL = junk_pool.tile([P, D], f32, tag="junk_act")
            nc.scalar.activation(
                out=jaL[:], in_=qL[:], func=Act.Square, accum_out=qqL[:],
            )
            # Full q-column epilogue for tile L, done at the head.
            qcol = 2 * L
            nc.vector.tensor_mul(nprod[:, qcol : qcol + 1], qqL[:], ddL[:])
            nc.scalar.activation(
                inv[:, qcol : qcol + 1], nprod[:, qcol : qcol + 1], Act.Sqrt
            )
            nc.vector.reciprocal(inv[:, qcol : qcol + 1], inv[:, qcol : qcol + 1])
            nc.vector.tensor_mul(
                contrib[:, qcol : qcol + 1], qdL[:], inv[:, qcol : qcol + 1]
            )
            nc.vector.tensor_mul(
                contrib[:, qcol : qcol + 1],
                contrib[:, qcol : qcol + 1],
                dvb[:, qcol : qcol + 1],
            )

            for t in range(T - 1):
                # Three 2 MB DMAs (d first) so compute can start as soon
                # as each component lands.
                d_t = data_pool.tile([P, D], f32, tag="d")
                q_t = data_pool.tile([P, D], f32, tag="q")
                a_t = data_pool.tile([P, D], f32, tag="a")
                nc.sync.dma_start(d_t[:], samples[bass.ts(t, P), 2, :])
                nc.sync.dma_start(q_t[:], samples[bass.ts(t, P), 0, :])
                nc.sync.dma_start(a_t[:], samples[bass.ts(t, P), 1, :])
                q, a, d = q_t[:], a_t[:], d_t[:]

                # DVE: fused product + per-partition accumulate.
                dve_accs = {}
                for src0, src1, col, atag in (
                    (d, d, None, "dd1"),
                    (q, d, 2 * t, "qd1"),
                    (a, d, 2 * t + 1, "ad1"),
                ):
                    jd = junk_pool.tile([P, D], f32, tag="junk_dve")
                    acc = junk_pool.tile([P, 1], f32, tag=atag)
                    nc.vector.scalar_tensor_tensor(
                        out=jd[:], in0=src0, scalar=1.0, in1=src1,
                        op0=Alu.mult, op1=Alu.mult, accum_out=acc[:],
                    )
                    dve_accs[atag] = acc
                    if col is not None:
                        nc.gpsimd.tensor_copy(dots[:, col : col + 1], acc[:])

                # ACT: square + accumulate for the q/a norms.
                for src0, col, atag in ((q, 2 * t, "qq1"), (a, 2 * t + 1, "aa1")):
                    ja = junk_pool.tile([P, D], f32, tag="junk_act")
                    acc = junk_pool.tile([P, 1], f32, tag=atag)
                    nc.scalar.activation(
                        out=ja[:], in_=src0, func=Act.Square, accum_out=acc[:],
                    )
                    nc.gpsimd.tensor_mul(
                        nprod[:, col : col + 1], acc[:], dve_accs["dd1"][:]
                    )

                # Per-tile epilogue: cos = dot * (1/sqrt(nprod)) * dvb.
                c2 = slice(2 * t, 2 * t + 2)
                nc.scalar.activation(inv[:, c2], nprod[:, c2], Act.Sqrt)
                nc.vector.reciprocal(inv[:, c2], inv[:, c2])
                nc.gpsimd.tensor_mul(contrib[:, c2], dots[:, c2], inv[:, c2])
                nc.gpsimd.tensor_mul(contrib[:, c2], contrib[:, c2], dvb[:, c2])

            # --- Tile L's `a` arrives last, in chunks. Dots on DVE with
            # running adds; squares of chunks 0-2 on ACT (hidden under
            # the stream); the final tiny chunk's square on DVE so the
            # scalar engine pre-loads the Sqrt table meanwhile.
            aL = stats_pool.tile([P, D], f32, tag="aL")
            ad_p = []
            aa_p = []
            nchunks = len(A_CHUNKS)
            for k, (c0, c1) in enumerate(A_CHUNKS):
                w = c1 - c0
                sl = slice(c0, c1)
                nc.sync.dma_start(aL[:, sl], samples[bass.ts(L, P), 1, sl])
                jd = junk_pool.tile([P, w], f32, tag=f"junk_adq_{w}")
                adk = junk_pool.tile([P, 1], f32, tag=f"ad_q{k}")
                nc.vector.scalar_tensor_tensor(
                    out=jd[:], in0=aL[:, sl], scalar=1.0, in1=dL[:, sl],
                    op0=Alu.mult, op1=Alu.mult, accum_out=adk[:],
                )
                aak = junk_pool.tile([P, 1], f32, tag=f"aa_q{k}")
                # ACT square (runs in parallel with DVE dots; Square
                # and Sqrt coexist in the runtime table, no reload).
                ja = junk_pool.tile([P, w], f32, tag=f"junk_aact_{w}")
                nc.scalar.activation(
                    out=ja[:], in_=aL[:, sl], func=Act.Square,
                    accum_out=aak[:],
                )
                ad_p.append(adk)
                aa_p.append(aak)

            # Pairwise tree adds on GpSimd: the 01 pair completes early
            # (hidden under the stream); only the 23 pair and the final
            # add sit behind the last chunk's work.
            ad01 = junk_pool.tile([P, 1], f32, tag="ad01")
            aa01 = junk_pool.tile([P, 1], f32, tag="aa01")
            ad23 = junk_pool.tile([P, 1], f32, tag="ad23")
            aa23 = junk_pool.tile([P, 1], f32, tag="aa23")
            ad_run = junk_pool.tile([P, 1], f32, tag="adT")
            aa_run = junk_pool.tile([P, 1], f32, tag="aaT")
            nc.gpsimd.tensor_add(ad01[:], ad_p[0][:], ad_p[1][:])
            nc.gpsimd.tensor_add(aa01[:], aa_p[0][:], aa_p[1][:])
            nc.gpsimd.tensor_add(ad23[:], ad_p[2][:], ad_p[3][:])
            nc.gpsimd.tensor_add(aa23[:], aa_p[2][:], aa_p[3][:])
            nc.gpsimd.tensor_add(ad_run[:], ad01[:], ad23[:])
            nc.gpsimd.tensor_add(aa_run[:], aa01[:], aa23[:])

            acol = 2 * L + 1
            nc.gpsimd.tensor_mul(nprod[:, acol : acol + 1], aa_run[:], ddL[:])
            nc.scalar.activation(
                inv[:, acol : acol + 1], nprod[:, acol : acol + 1], Act.Sqrt
            )
            nc.vector.reciprocal(inv[:, acol : acol + 1], inv[:, acol : acol + 1])
            # contrib_a = ad * dvb_a * inv_a in ONE fused stt.
            nc.vector.scalar_tensor_tensor(
                out=contrib[:, acol : acol + 1], in0=ad_run[:], scalar=dvb_a[:],
                op0=Alu.mult, op1=Alu.mult, in1=inv[:, acol : acol + 1],
            )

            # Row sum, then partition reduce with one [1,1] fp32 matmul.
            row_sum = stats_pool.tile([P, 1], f32, tag="row_sum")
            nc.vector.reduce_sum(row_sum[:], contrib[:], axis=mybir.AxisListType.X)
            psum_t = psum_pool.tile([1, 1], f32, tag="psum_s")
            nc.tensor.matmul(psum_t[:], row_sum[:], ones[:], start=True, stop=True)

            partial = stats_pool.tile([1, 1], f32, tag="partial")
            nc.vector.tensor_copy(partial[:], psum_t[:])

            cc_in = dram_pool.tile([1, 1], f32, tag="cc_in")
            cc_out = dram_pool.tile([1, 1], f32, tag="cc_out")
            nc.sync.dma_start(cc_in[:], partial[:])

            # Pre-load the Exp table while the AllReduce is in flight.
            exp_warm = stats_pool.tile([1, 1], f32, tag="exp_warm")
            nc.scalar.activation(exp_warm[:], ones[0:1, 0:1], Act.Exp)

            nc.gpsimd.collective_compute(
                "AllReduce",
                Alu.add,
                replica_groups=[list(range(N_CORES))],
                ins=[cc_in[:].opt()],
                outs=[cc_out[:].opt()],
            )
            red = stats_pool.tile([1, 1], f32, tag="red")
            nc.sync.dma_start(red[:], cc_out[:])
            s = red[0:1, 0:1]

            # BCE = ln(1+exp(s)) - s*y (safe for |s| << 88; score O(5)).
            exp_t = stats_pool.tile([1, 1], f32, tag="exp_t")
            sp_t = stats_pool.tile([1, 1], f32, tag="sp_t")
            bce_t = stats_pool.tile([1, 1], f32, tag="bce_t")
            nc.scalar.activation(exp_t[:], s, Act.Exp)
            nc.scalar.activation(sp_t[:], exp_t[:], Act.Ln, bias=1.0)
            nc.vector.scalar_tensor_tensor(
                out=bce_t[:], in0=s, scalar=lneg[:], in1=sp_t[:],
                op0=Alu.mult, op1=Alu.add,
            )

            nc.sync.dma_start(out[:], bce_t[:])

    nc.compile()
    return nc


def _get_program():
    if "nc" not in _CACHE:
        _CACHE["nc"] = _build_program()
    return _CACHE["nc"]


def kernel(samples, labels, D_v1, D_v2):
    samples = np.asarray(samples, dtype=np.float32)
    labels = np.asarray(labels, dtype=np.float32)
    D_v1 = np.asarray(D_v1, dtype=np.float32)
    D_v2 = np.asarray(D_v2, dtype=np.float32)
    assert samples.shape == (B, 3, D), samples.shape

    nc = _get_program()

    in_maps = []
    for c in range(N_CORES):
        sl = slice(c * BS, (c + 1) * BS)
        in_maps.append(
            {
                "samples": np.ascontiguousarray(samples[sl]),
                "labels": labels,
                "dv1": np.ascontiguousarray(D_v1[sl]),
                "dv2": np.ascontiguousarray(D_v2[sl]),
            }
        )

    _tc = os.environ.get("KERNEL_TRACE_CORES")
    _kw = {"trace_cores": [int(x) for x in _tc.split(",")]} if _tc else {}
    try:
        res = bass_utils.run_bass_kernel_spmd(
            nc, in_maps, core_ids=list(range(N_CORES)), **_kw
        )
    except Exception:
        # A previously-wedged NeuronCore surfaces as an unrecoverable
        # exec error on the first attempt; the runtime resets it, so a
        # single retry recovers.
        res = bass_utils.run_bass_kernel_spmd(
            nc, in_maps, core_ids=list(range(N_CORES)), **_kw
        )
    _CACHE["last_results"] = res
    return np.asarray(res.results[0]["out"], dtype=np.float32).reshape(())
